# revision 45
# baseline (speedup 1.0000x reference)
"""BiLSTM-CRF NLL kernel for 8 Trainium2 NeuronCores.

Contract: kernel(**inputs) takes the FULL unsharded inputs (as produced by the
reference setup_inputs()) and returns the FULL output (a float32 scalar).

Sharding strategy (hardcoded): data-parallel over the batch dim. B=64 is split
into 8 shards of 8 sequences; LSTM/CRF parameters are replicated on every core.
Each core computes the total NLL of its 8 sequences on-device; the host sums
the 8 partial scalars (the "unshard" step).

Per-core pipeline (all on device):
  0. embedding gather via indirect DMA (token-major [128, E] tiles),
     PE transposes to xT [E, tokens]
  1. input projections g_ih = W_ih @ x + b for all tokens as dense matmuls
     (f32), stored fp16 in SBUF, gate chunks pre-permuted to (i,i,f,f,o,o,g,g)
  2. the two LSTM recurrences (fwd / bwd), interleaved with a half-step skew.
     Per step: 16 bf16 [128,128] weight tiles x [128,8] h -> PSUM [128,64],
     plus one identity-matmul that accumulates the precomputed g_ih into the
     same PSUM bank.  The whole nonlinear tail runs as FOUR back-to-back DVE
     instructions (two custom ops):
       SIG5:  s' = sig~(ps) - 0.5      (deg-5 odd poly + clamp, all 4 gates;
                                        g-gate pre-scaled x2 so s'_g=tanh(g)/2)
       stt:   [u|v|so] = (s'_{i,f,o}+0.5) * [g'|c_old|1]
       stt:   c_new    = 2u + v
       PHM:   h        = tanh~(c_new) * so   (deg-3 odd poly + clamp)
     No ACT/GPSIMD in the chain -> the serial tail latency drops ~2x, and the
     f/b skew hides each direction's tail under the other's matmul group.
  3. emissions transposed [9, tokens] = W_tag.T-chunks @ h, E = exp(emis - mu)
  4. CRF in exp space: the forward-algorithm logsumexp becomes
     A' = (exp(trans).T @ A) * E_t  -- a [9,9]x[9,8] matmul plus one
     elementwise multiply per step. Meet-in-the-middle: a forward chain
     (t=0..127) and a backward chain (t=255..128) run concurrently, halving
     the sequential depth; logZ = log(sum_i A_127 * B_127) + 256*mu.
  5. gold path score via one-hot tensors (host-encoded from tags) and
     matmuls/reductions; output = sum_b (logZ_b - score_b) as [1,1] f32.
"""

import functools
import math
import os
import sys

import numpy as np

for _p in ("/opt/trn_rl_repo", "/opt/pypackages"):
    if _p not in sys.path and os.path.isdir(_p):
        sys.path.append(_p)

import ml_dtypes  # noqa: E402

import concourse.bass as bass  # noqa: E402
import concourse.mybir as mybir  # noqa: E402
import concourse.tile as tile  # noqa: E402
from concourse import bacc  # noqa: E402
from concourse.bass import IndirectOffsetOnAxis  # noqa: E402
from concourse.bass_utils import run_bass_kernel_spmd  # noqa: E402

F32 = mybir.dt.float32
F16 = mybir.dt.float16
BF16 = mybir.dt.bfloat16
I32 = mybir.dt.int32
AF = mybir.ActivationFunctionType
OP = mybir.AluOpType

# Problem constants (hardcoded per the task contract).
B, S, V, E, H, T = 64, 256, 50000, 256, 512, 9
HD = H // 2               # 256 per-direction hidden
NCORES = 8
BL = B // NCORES          # 8 sequences per core
TOK = BL * S              # 2048 tokens per core
NCH = TOK // 128          # 16 gather chunks of 128 tokens
MU = math.log(9.0)        # exp-space drift compensation, cancels exactly
# gate chunk permutation: original (i0 i1 f0 f1 g0 g1 o0 o1) -> (i i f f o o g g)
PERM = [0, 1, 2, 3, 6, 7, 4, 5]
HSLOT = 16                # one h slot = 2 hd-chunks x 8 batch

# --- polynomial activation approximations (fit offline, minimax-ish) -------
# sig~(x)-0.5 = z*(A0 + z^2*(A1 + A2*z^2)), z = clamp(x, +-SIG_B)
SIG_B = 4.2966
SIG_A0 = 0.23636622
SIG_A1 = -0.0121209
SIG_A2 = 0.00030062
# tanh~(x) = z*(T0 + T1*z^2), z = clamp(x, +-TANH_B)
TANH_B = 1.5789
TANH_T0 = 0.89413763
TANH_T1 = -0.12256313


def _register_dve_ops():
    """Register the two LSTM-tail custom DVE ops (idempotent)."""
    from concourse import dve_ops as dops
    from concourse.dve_ops import (DveOp, DveOpSpec, get_dve_sub_opcode,
                                   has_src1)
    from concourse.dve_spec import (  # noqa: F401
        Spec, Src0, Src1, C0, C1, C2, C3, Zero, maxx, minn, sq, lower,
        _spill_c3_to_src1,
    )

    def reg(name, spec):
        for op in dops.OPS:
            if op.name == name:
                return op
        op = DveOp(name, spec, subdim=False, uops_sha={})
        dops.OPS.append(op)
        dops.CUSTOM_DVE_SPECS[name] = spec
        dops._SUB_OPCODE_FOR_NAME[name] = (
            dops._CUSTOM_DVE_ROW_BASE + len(dops.OPS) - 1
        )
        for ver in ("v3", "v4"):
            try:
                compiled = DveOpSpec(
                    name=name, opcode=get_dve_sub_opcode(name),
                    uops=lower(spec, ver=ver), rd1_en=has_src1(spec),
                )
                op.uops_sha[ver] = compiled.sha(ver)
            except Exception:
                pass
        return op

    # SIG5: out = z*(s1 + z2*(imm2 + c3*z2)), z = clamp(in0, +-s0); c3 via in1
    # (minn first so the hoisted Zero-C0 latch-read lands deeper than stage 0)
    z = maxx(minn(Src0, C0), Zero - C0)
    z2 = sq(z)
    w = ((z2 * C3) + C2) * z2 + C1
    sig_body = _spill_c3_to_src1(w * z)

    def sig_ref(in0, in1, s0, s1, imm2):
        zz = np.clip(in0, -s0, s0)
        return (zz * (s1 + zz * zz * (imm2 + in1 * zz * zz))).astype(np.float32)

    sig5 = reg("LSTM_SIG5_ANT", Spec(body=sig_body, reference=sig_ref))

    # PHM: out = in1 * z*(s1 + imm2*z^2), z = clamp(in0, +-s0)
    zp = maxx(minn(Src0, C0), Zero - C0)
    tp = ((sq(zp) * C2) + C1) * zp
    phm_body = tp * Src1

    def phm_ref(in0, in1, s0, s1, imm2):
        zz = np.clip(in0, -s0, s0)
        return (in1 * zz * (s1 + imm2 * zz * zz)).astype(np.float32)

    phm = reg("LSTM_PHM_ANT", Spec(body=phm_body, reference=phm_ref))

    # SIG3T: out = z*(s1 + imm2*z^2), z = clamp(in0 + in1, +-s0).
    # in1 carries the precomputed g_ih tile, so no PSUM preload matmul is
    # needed: the W_hh matmuls alone produce in0.
    z3 = maxx(minn(Src0 + Src1, C0), Zero - C0)
    sig3_body = ((sq(z3) * C2) + C1) * z3

    def sig3_ref(in0, in1, s0, s1, imm2):
        zz = np.clip(in0 + in1, -s0, s0)
        return (zz * (s1 + imm2 * zz * zz)).astype(np.float32)

    sig3 = reg("LSTM_SIG3T_ANT", Spec(body=sig3_body, reference=sig3_ref))
    return sig5, phm, sig3


SIG5_OP, PHM_OP, SIG3_OP = _register_dve_ops()

# deg-3 sigmoid coefficients (fit on [0, 8.3], clamp at SIG3_B)
SIG3_B = 3.3590
SIG3_A0 = 0.21648028
SIG3_A1 = -0.00647874


def _emit_preload(nc, d, t, gih, idf16, ps_pool):
    """Start step-t PSUM with g_ih (+bias) via identity matmul (h-independent)."""
    ps = ps_pool[d].tile([128, 64], F32, tag=f"st{d}", name=f"ps{d}")
    nc.tensor.matmul(
        out=ps[:, :],
        lhsT=idf16[:],
        rhs=gih[d][:, t * 64:(t + 1) * 64],
        start=True,
        stop=False,
        skip_group_check=True,
    )
    return ps


def _emit_wmms(nc, d, t, whh, hall, ps_pool):
    """16 W_hh matmuls for one direction-step; allocates and returns the
    PSUM tile (no preload: g_ih is added later inside the SIG3T DVE op)."""
    ps = ps_pool[d].tile([128, 64], F32, tag=f"st{d}", name=f"ps{d}")
    rd = t if d == "f" else t + 1
    for m in range(8):
        for k in range(2):
            nc.tensor.matmul(
                out=ps[:, m * 8:(m + 1) * 8],
                lhsT=whh[d][k][:, m * 128:(m + 1) * 128],
                rhs=hall[d][:, rd * HSLOT + k * 8: rd * HSLOT + k * 8 + 8],
                start=(k == 0),
                stop=(m == 7 and k == 1),
                skip_group_check=True,
            )
    return ps


def _emit_tail_merged(nc, t, ps_cur, hall, arena, work, gih, sig_only=None):
    """LSTM nonlinear tail for BOTH directions as 6 DVE instructions.

    Arena layout per parity buffer [128, 192]: direction d at base D (f=0,
    b=96): [D:D+64] sig outputs, [D+64:D+80] c state, [D+80:D+96] const 1.
    The cell-update stt ops cover both directions at once through
    [128, 2, 48]-strided views.
    """
    k, k2 = t % 2, (t + 1) % 2

    def tail_one(d, base):
        cur, nxt = arena[k], arena[k2]
        tok = t if d == "f" else S - 1 - t
        nc.vector._custom_dve(
            SIG3_OP, out=cur[:, base:base + 64], in0=ps_cur[d][:, :],
            in1=gih[d][:, tok * 64:(tok + 1) * 64],
            s0=SIG3_B, s1=SIG3_A0, imm2=SIG3_A1,
        )
        uvo = work.tile([128, 48], F32, tag=f"uvo{d}", name=f"uvo{d}")
        nc.vector.scalar_tensor_tensor(
            uvo[:], cur[:, base:base + 48], 0.5,
            cur[:, base + 48:base + 96], op0=OP.add, op1=OP.mult,
        )
        nc.vector.scalar_tensor_tensor(
            nxt[:, base + 64:base + 80], uvo[:, 0:16], 2.0, uvo[:, 16:32],
            op0=OP.mult, op1=OP.add,
        )
        wr = t + 1 if d == "f" else S - 1 - t
        nc.vector._custom_dve(
            PHM_OP, out=hall[d][:, wr * HSLOT:(wr + 1) * HSLOT],
            in0=nxt[:, base + 64:base + 80], in1=uvo[:, 32:48],
            s0=TANH_B, s1=TANH_T0, imm2=TANH_T1,
        )

    if sig_only == "f":
        tail_one("f", 0)
        return
    tail_one("b", 96)


@functools.lru_cache(maxsize=2)
def _build(seq_len=S):
    """Build the Bass program (same SPMD program for all 8 cores)."""
    assert seq_len == S, "builder is specialized to S=256"

    nc = bacc.Bacc("TRN2", target_bir_lowering=False, debug=False)

    # ---- DRAM I/O ----
    emb_d = nc.dram_tensor("emb", [V, E], F32, kind="ExternalInput")
    idx_d = nc.dram_tensor("idx", [128, NCH], I32, kind="ExternalInput")
    wih_d = {d: nc.dram_tensor(f"wih_{d}", [E, 4 * HD], F32, kind="ExternalInput")
             for d in "fb"}
    whh_d = {d: nc.dram_tensor(f"whh_{d}", [HD, 4 * HD], BF16, kind="ExternalInput")
             for d in "fb"}
    br_d = {d: nc.dram_tensor(f"br_{d}", [128, 8], F32, kind="ExternalInput")
            for d in "fb"}
    wtag_d = nc.dram_tensor("wtagT", [H, T], BF16, kind="ExternalInput")
    btag_d = nc.dram_tensor("btag", [T, 1], F32, kind="ExternalInput")
    start_d = nc.dram_tensor("startv", [T, 1], F32, kind="ExternalInput")
    end_d = nc.dram_tensor("endv", [T, 1], F32, kind="ExternalInput")
    trans_d = nc.dram_tensor("transm", [T, T], F32, kind="ExternalInput")
    transT_d = nc.dram_tensor("transmT", [T, T], F32, kind="ExternalInput")
    ohc_d = nc.dram_tensor("ohc", [T, TOK], F32, kind="ExternalInput")
    ohn_d = nc.dram_tensor("ohn", [T, TOK], F32, kind="ExternalInput")
    idf32_d = nc.dram_tensor("idf32", [128, 128], F32, kind="ExternalInput")
    idf16_d = nc.dram_tensor("idf16", [128, 128], F16, kind="ExternalInput")
    idq_d = nc.dram_tensor("idq", [T, 72], F32, kind="ExternalInput")
    out_d = nc.dram_tensor("out", [1, 1], F32, kind="ExternalOutput")

    with tile.TileContext(nc) as tc:
        with (
            tc.tile_pool(name="pers", bufs=1) as pers,
            tc.tile_pool(name="work", bufs=3) as work,
            tc.tile_pool(name="psbig", bufs=2, space="PSUM") as ps_big,
            tc.tile_pool(name="pstp", bufs=2, space="PSUM") as ps_tp,
            tc.tile_pool(name="psf", bufs=2, space="PSUM") as ps_f,
            tc.tile_pool(name="psb", bufs=2, space="PSUM") as ps_b,
        ):
            ps_pool = {"f": ps_f, "b": ps_b}

            # ---- persistent SBUF ----
            idx_sb = pers.tile([128, NCH], I32, tag="idx")
            nc.sync.dma_start(idx_sb[:], idx_d[:])
            idf32 = pers.tile([128, 128], F32, tag="idf32")
            nc.sync.dma_start(idf32[:], idf32_d[:])
            idf16 = pers.tile([128, 128], F16, tag="idf16")
            nc.sync.dma_start(idf16[:], idf16_d[:])
            a2t = pers.tile([128, 1], F32, tag="a2t")
            nc.vector.memset(a2t[:], SIG_A2)
            a2b = pers.tile([128, 1], F32, tag="a2b")
            nc.vector.memset(a2b[:], SIG_A2)

            wih, whh, br, gih, hall = {}, {}, {}, {}, {}
            for d in "fb":
                wih[d] = [pers.tile([128, 4 * HD], F32, tag=f"wih{d}{k}",
                                    name=f"wih{d}{k}") for k in range(2)]
                for k in range(2):
                    nc.sync.dma_start(wih[d][k][:], wih_d[d][k * 128:(k + 1) * 128, :])
                whh[d] = [pers.tile([128, 4 * HD], BF16, tag=f"whh{d}{k}",
                                    name=f"whh{d}{k}") for k in range(2)]
                for k in range(2):
                    nc.sync.dma_start(whh[d][k][:], whh_d[d][k * 128:(k + 1) * 128, :])
                br[d] = pers.tile([128, 8], F32, tag=f"br{d}", name=f"br{d}")
                nc.sync.dma_start(br[d][:], br_d[d][:])
                gih[d] = pers.tile([128, S * 64], F16, tag=f"gih{d}", name=f"gih{d}")
                hall[d] = pers.tile([128, (S + 1) * HSLOT], BF16, tag=f"hall{d}", name=f"hall{d}")
            # merged tail arenas (both directions in one tile, 2 parities)
            arena = [pers.tile([128, 192], F32, tag=f"arm{k}", name=f"arm{k}")
                     for k in range(2)]
            for base in (0, 96):
                nc.vector.memset(arena[0][:, base + 64:base + 80], 0.0)
                nc.vector.memset(arena[0][:, base + 80:base + 96], 1.0)
                nc.vector.memset(arena[1][:, base + 80:base + 96], 1.0)

            # zero initial h slots (fwd reads slot 0, bwd reads slot S)
            nc.vector.memset(hall["f"][:, 0:HSLOT], 0.0)
            nc.vector.memset(hall["b"][:, S * HSLOT:(S + 1) * HSLOT], 0.0)

            wtagT = [pers.tile([128, T], BF16, tag=f"wtag{kk}", name=f"wtag{kk}")
                      for kk in range(4)]
            for kk in range(4):
                nc.sync.dma_start(wtagT[kk][:], wtag_d[kk * 128:(kk + 1) * 128, :])
            btag = pers.tile([T, 1], F32, tag="btag")
            nc.sync.dma_start(btag[:], btag_d[:])
            startv = pers.tile([T, 1], F32, tag="startv")
            nc.sync.dma_start(startv[:], start_d[:])
            endv = pers.tile([T, 1], F32, tag="endv")
            nc.sync.dma_start(endv[:], end_d[:])
            transm = pers.tile([T, T], F32, tag="transm")
            nc.sync.dma_start(transm[:], trans_d[:])
            transmT = pers.tile([T, T], F32, tag="transmT")
            nc.sync.dma_start(transmT[:], transT_d[:])
            ohc = pers.tile([T, TOK], F32, tag="ohc")
            nc.sync.dma_start(ohc[:], ohc_d[:])
            ohn = pers.tile([T, TOK], F32, tag="ohn")
            nc.sync.dma_start(ohn[:], ohn_d[:])
            ones9 = pers.tile([T, 1], F32, tag="ones9")
            nc.vector.memset(ones9[:], 1.0)
            ones98 = pers.tile([T, 8], F32, tag="ones98")
            nc.vector.memset(ones98[:], 1.0)
            idq = pers.tile([T, 72], F32, tag="idq")
            nc.sync.dma_start(idq[:], idq_d[:])

            # ---- phase 0: gather all chunks up-front (one serial DMA queue,
            # interleaved fwd/bwd order); transposes + per-chunk phase-1 are
            # emitted INSIDE the step loop so the PE FIFO never blocks on a
            # late gather.
            xg = pers.tile([128, NCH * E], F32, tag="xg")
            xT = [pers.tile([128, TOK], F32, tag=f"xT{k}", name=f"xT{k}")
                  for k in range(2)]
            gorder = []
            for j in range(NCH // 2):
                gorder += [j, NCH - 1 - j]
            for ch in gorder:
                nc.gpsimd.indirect_dma_start(
                    out=xg[:, ch * E:(ch + 1) * E],
                    out_offset=None,
                    in_=emb_d[:],
                    in_offset=IndirectOffsetOnAxis(ap=idx_sb[:, ch:ch + 1], axis=0),
                )

            transposed = set()

            def emit_transpose(ch):
                if ch in transposed:
                    return
                transposed.add(ch)
                for k in range(2):
                    pst = ps_tp.tile([128, 128], F32, tag="tp", name="tp")
                    nc.tensor.transpose(
                        out=pst[:],
                        in_=xg[:, ch * E + k * 128: ch * E + (k + 1) * 128],
                        identity=idf32[:],
                    )
                    nc.scalar.copy(xT[k][:, ch * 128:(ch + 1) * 128], pst[:])

            def emit_phase1(d, ch):
                # input projections for one 128-token chunk of direction d.
                # All bias-add/copy ops routed OFF the vector engine (it is
                # reserved for the recurrence tail chain).
                emit_transpose(ch)
                for m in range(8):
                    psg = ps_big.tile([128, 128], F32, tag="big", name="psg")
                    for k in range(2):
                        nc.tensor.matmul(
                            out=psg[:],
                            lhsT=wih[d][k][:, m * 128:(m + 1) * 128],
                            rhs=xT[k][:, ch * 128:(ch + 1) * 128],
                            start=(k == 0),
                            stop=(k == 1),
                        )
                    dst = gih[d][:].rearrange(
                        "p (t m b) -> p t m b", t=S, m=8, b=8
                    )[:, ch * 16:(ch + 1) * 16, m, :]
                    srcv = psg[:].rearrange("p (t b) -> p t b", t=16, b=8)
                    nc.scalar.activation(dst, srcv, AF.Identity,
                                         bias=br[d][:, m:m + 1])

            # ---- phase 1+2 interleaved, with a half-step skew between the
            # two directions: direction f's tail (DVE) executes while
            # direction b's W-matmuls run on the PE, and vice versa.
            emit_phase1("f", 0)
            emit_phase1("b", NCH - 1)
            ps_cur = {"f": _emit_wmms(nc, "f", 0, whh, hall, ps_pool)}
            for t in range(S):
                if t == 8:
                    emit_phase1("f", 1)
                elif t >= 16 and t % 16 == 0:
                    q = t // 16
                    if q + 1 < NCH:
                        emit_phase1("f", q + 1)
                tokb = S - 1 - t
                _emit_tail_merged(nc, t, ps_cur, hall, arena, work, gih,
                                  sig_only="f")
                ps_cur["b"] = _emit_wmms(nc, "b", tokb, whh, hall, ps_pool)
                if t == 8:
                    emit_phase1("b", NCH - 2)
                elif t >= 16 and t % 16 == 0:
                    q = t // 16
                    if NCH - 2 - q >= 0:
                        emit_phase1("b", NCH - 2 - q)
                _emit_tail_merged(nc, t, ps_cur, hall, arena, work, gih)
                if t + 1 < S:
                    ps_cur["f"] = _emit_wmms(nc, "f", t + 1, whh, hall,
                                             ps_pool)

            # ---- phase 3: emissions (transposed) + E = exp(emis - mu) ----
            emisraw = pers.tile([T, TOK], F32, tag="emisraw")
            ebuf = pers.tile([T, TOK], F32, tag="ebuf")
            hview = {d: hall[d][:].rearrange("p (s c b) -> p s c b", s=S + 1, c=2, b=8)
                     for d in "fb"}
            for n in (1, 2, 0, 3):
                pse = ps_big.tile([T, 512], F32, tag="big")
                for kk in range(4):
                    d = "f" if kk < 2 else "b"
                    c = kk % 2
                    lo = n * 64 + (1 if d == "f" else 0)
                    rhs = hview[d][:, lo:lo + 64, c, :]
                    nc.tensor.matmul(
                        out=pse[:],
                        lhsT=wtagT[kk][:],
                        rhs=rhs,
                        start=(kk == 0),
                        stop=(kk == 3),
                    )
                nc.vector.tensor_scalar_add(
                    emisraw[:, n * 512:(n + 1) * 512], pse[:], btag[:, 0:1]
                )
            negmu = pers.tile([T, 1], F32, tag="negmu")
            nc.vector.memset(negmu[:], -MU)
            nc.scalar.activation(ebuf[:], emisraw[:], AF.Exp, bias=negmu[:, 0:1])

            # ---- phase 4: gold path score ----
            tmp9 = pers.tile([T, TOK], F32, tag="tmp9")
            nc.vector.tensor_tensor(tmp9[:], emisraw[:], ohc[:], op=OP.mult)
            gm = pers.tile([T, 8], F32, tag="gm")
            nc.vector.tensor_reduce(
                gm[:],
                tmp9[:].rearrange("p (t b) -> p b t", t=S, b=8),
                axis=mybir.AxisListType.X,
                op=OP.add,
            )
            for n in range(4):
                psg2 = ps_big.tile([T, 512], F32, tag="big")
                nc.tensor.matmul(
                    out=psg2[:],
                    lhsT=transm[:],
                    rhs=ohc[:, n * 512:(n + 1) * 512],
                    start=True,
                    stop=True,
                )
                nc.vector.tensor_tensor(
                    tmp9[:, n * 512:(n + 1) * 512], psg2[:],
                    ohn[:, n * 512:(n + 1) * 512], op=OP.mult,
                )
            gtr = pers.tile([T, 8], F32, tag="gtr")
            nc.vector.tensor_reduce(
                gtr[:],
                tmp9[:].rearrange("p (t b) -> p b t", t=S, b=8),
                axis=mybir.AxisListType.X,
                op=OP.add,
            )
            gse = pers.tile([T, 8], F32, tag="gse")
            nc.vector.tensor_scalar(
                gse[:], ohc[:, 0:8], scalar1=startv[:, 0:1], scalar2=None,
                op0=OP.mult,
            )
            gee = pers.tile([T, 8], F32, tag="gee")
            nc.vector.tensor_scalar(
                gee[:], ohc[:, (S - 1) * 8:S * 8], scalar1=endv[:, 0:1],
                scalar2=None, op0=OP.mult,
            )
            nc.vector.tensor_tensor(gm[:], gm[:], gtr[:], op=OP.add)
            nc.vector.tensor_tensor(gse[:], gse[:], gee[:], op=OP.add)
            nc.vector.tensor_tensor(gm[:], gm[:], gse[:], op=OP.add)
            ps_sc = ps_tp.tile([1, 8], F32, tag="tp")
            nc.tensor.matmul(out=ps_sc[:], lhsT=ones9[:], rhs=gm[:],
                             start=True, stop=True)
            score_sb = pers.tile([1, 8], F32, tag="score")
            nc.vector.tensor_copy(score_sb[:], ps_sc[:])

            # ---- phase 5: CRF forward/backward exp-space chains ----
            expT = pers.tile([T, T], F32, tag="expT")
            nc.scalar.activation(expT[:], transm[:], AF.Exp)
            expTT = pers.tile([T, T], F32, tag="expTT")
            nc.scalar.activation(expTT[:], transmT[:], AF.Exp)
            exps = pers.tile([T, 1], F32, tag="exps")
            nc.scalar.activation(exps[:], startv[:], AF.Exp)
            expe = pers.tile([T, 1], F32, tag="expe")
            nc.scalar.activation(expe[:], endv[:], AF.Exp)

            # 4-chain scan: cols 0:8 = A (fwd from t=0), 8:16 = B (bwd from
            # t=255), 16:88 = X (identity-init, fwd t=65..127), 88:160 = Y
            # (identity-init, bwd t=190..128).  X/Y columns are b-major
            # (col = 16/88 + b*9 + i).  63 iterations instead of 127; the
            # quarter-chains are stitched with PE-transpose + block-diag
            # matmuls afterwards.
            e3 = ebuf[:].rearrange("p (t b) -> p t b", t=S, b=8)
            tmpAB = work.tile([T, 16], F32, tag="tmpAB")
            nc.vector.tensor_scalar(
                tmpAB[:, 0:8], ebuf[:, 0:8], scalar1=exps[:, 0:1], scalar2=None,
                op0=OP.mult,
            )
            nc.vector.tensor_scalar(
                tmpAB[:, 8:16], ebuf[:, (S - 1) * 8:S * 8],
                scalar1=expe[:, 0:1], scalar2=None, op0=OP.mult,
            )
            # identity init for X and Y: I9 replicated per sequence (host input)
            tmpX = work.tile([T, 72], F32, tag="tmpX")
            tmpY = work.tile([T, 72], F32, tag="tmpY")
            nc.vector.tensor_copy(tmpX[:], idq[:])
            nc.vector.tensor_copy(tmpY[:], idq[:])
            NQ = 63
            for i in range(NQ):  # A: t=1..63; B: t=254..192; X: 65..127; Y: 190..128
                tA = 1 + i
                tB = S - 2 - i
                tX = 65 + i
                tY = S - 66 - i
                psAB = ps_f.tile([T, 16], F32, tag="stf", name="psAB")
                nc.tensor.matmul(out=psAB[:, 0:8], lhsT=expT[:],
                                 rhs=tmpAB[:, 0:8], start=True, stop=True)
                nc.tensor.matmul(out=psAB[:, 8:16], lhsT=expTT[:],
                                 rhs=tmpAB[:, 8:16], start=True, stop=True)
                psX = ps_b.tile([T, 72], F32, tag="stb", name="psX")
                nc.tensor.matmul(out=psX[:], lhsT=expT[:],
                                 rhs=tmpX[:], start=True, stop=True)
                psY = ps_big.tile([T, 72], F32, tag="big", name="psY")
                nc.tensor.matmul(out=psY[:], lhsT=expTT[:],
                                 rhs=tmpY[:], start=True, stop=True)
                tmpAB = work.tile([T, 16], F32, tag="tmpAB")
                nc.vector.tensor_tensor(
                    tmpAB[:], psAB[:],
                    e3[:, tA:tB + 1:(tB - tA), :], op=OP.mult,
                )
                tmpX = work.tile([T, 72], F32, tag="tmpX")
                nc.vector.tensor_tensor(
                    tmpX[:].rearrange("p (b i) -> p b i", b=8, i=9),
                    psX[:].rearrange("p (b i) -> p b i", b=8, i=9),
                    e3[:, tX, :].broadcast_to([T, 8, 9]), op=OP.mult,
                )
                tmpY = work.tile([T, 72], F32, tag="tmpY")
                nc.vector.tensor_tensor(
                    tmpY[:].rearrange("p (b i) -> p b i", b=8, i=9),
                    psY[:].rearrange("p (b i) -> p b i", b=8, i=9),
                    e3[:, tY, :].broadcast_to([T, 8, 9]), op=OP.mult,
                )
            # bridges: A_64 = (expT^T A_63) * E_64 ; B_191 = (expTT^T B_192)*E_191
            psbr = ps_f.tile([T, 16], F32, tag="stf", name="psbr")
            nc.tensor.matmul(out=psbr[:, 0:8], lhsT=expT[:],
                             rhs=tmpAB[:, 0:8], start=True, stop=True)
            nc.tensor.matmul(out=psbr[:, 8:16], lhsT=expTT[:],
                             rhs=tmpAB[:, 8:16], start=True, stop=True)
            ab64 = pers.tile([T, 16], F32, tag="ab64")
            nc.vector.tensor_tensor(
                ab64[:], psbr[:],
                e3[:, 64:192:127, :], op=OP.mult,
            )
            # stitch: A_127 = X_blockdiag @ A_64, B_128st = Y_blockdiag @ B_191
            abf = pers.tile([T, 16], F32, tag="abf")
            for side, src in (("x", tmpX), ("y", tmpY)):
                pstx = ps_tp.tile([128, 128], F32, tag="tp", name="tp")
                nc.tensor.transpose(out=pstx[0:72, 0:T],
                                    in_=src[:],
                                    identity=idf32[0:T, 0:T])
                xt = pers.tile([72, T], F32, tag=f"xt{side}")
                nc.vector.tensor_copy(xt[:], pstx[0:72, 0:T])
                bd = pers.tile([72, 8], F32, tag=f"bd{side}")
                nc.vector.memset(bd[:], 0.0)
                co = 0 if side == "x" else 8
                for bq in range(8):
                    nc.sync.dma_start(bd[bq * 9:bq * 9 + 9, bq:bq + 1],
                                      ab64[:, co + bq:co + bq + 1])
                psf2 = ps_tp.tile([T, 8], F32, tag="tp", name="tpf")
                nc.tensor.matmul(out=psf2[:], lhsT=xt[:], rhs=bd[:],
                                 start=True, stop=True)
                nc.vector.tensor_copy(abf[:, co:co + 8], psf2[:])
            # final B matmul: bridge the middle transition t=128
            psB = ps_b.tile([T, 8], F32, tag="stb")
            nc.tensor.matmul(out=psB[:], lhsT=expTT[:], rhs=abf[:, 8:16],
                             start=True, stop=True)
            ab = work.tile([T, 8], F32, tag="ab")
            nc.vector.tensor_tensor(ab[:], abf[:, 0:8], psB[:], op=OP.mult)
            psZ = ps_tp.tile([1, 8], F32, tag="tp")
            nc.tensor.matmul(out=psZ[:], lhsT=ones9[:], rhs=ab[:],
                             start=True, stop=True)
            lz = pers.tile([1, 8], F32, tag="lz")
            nc.scalar.activation(lz[:], psZ[:], AF.Ln)
            diff = pers.tile([1, 8], F32, tag="diff")
            nc.vector.tensor_tensor(diff[:], lz[:], score_sb[:], op=OP.subtract)
            red = pers.tile([1, 1], F32, tag="red")
            nc.vector.tensor_reduce(red[:], diff[:], axis=mybir.AxisListType.X,
                                    op=OP.add)
            outc = pers.tile([1, 1], F32, tag="outc")
            nc.vector.tensor_scalar_add(outc[:], red[:], float(BL * S * MU))
            nc.sync.dma_start(out_d[:], outc[:])

    nc.finalize()
    return nc


def _prep_inputs(x, tags, crf_mask, embedding, W_ih_f, W_hh_f, b_f, W_ih_b,
                 W_hh_b, b_b, W_tag, b_tag, transitions, start_trans, end_trans):
    """Host-side sharding + layout prep. Pure reformatting / dtype casts."""
    x = np.asarray(x).astype(np.int32)
    tags = np.asarray(tags).astype(np.int32)
    mask = np.asarray(crf_mask)
    assert mask.all(), "kernel specialized to all-ones crf_mask"
    embedding = np.ascontiguousarray(np.asarray(embedding, dtype=np.float32))

    def perm_cols(w):  # [*, 4HD] -> gate-chunk permuted cols, g-gate x2
        wc = w.reshape(w.shape[0], 8, 128)[:, PERM, :].copy()
        wc[:, 6:8, :] *= 2.0  # g-gate pre-scale: tanh(g) = 2*sigmoid(2g) - 1
        return np.ascontiguousarray(wc.reshape(w.shape[0], 4 * HD))

    wih = {"f": perm_cols(np.asarray(W_ih_f, np.float32).T),
           "b": perm_cols(np.asarray(W_ih_b, np.float32).T)}
    whh = {"f": perm_cols(np.asarray(W_hh_f, np.float32).T).astype(ml_dtypes.bfloat16),
           "b": perm_cols(np.asarray(W_hh_b, np.float32).T).astype(ml_dtypes.bfloat16)}
    brs = {}
    for d, b_ in (("f", b_f), ("b", b_b)):
        bv = np.asarray(b_, np.float32).reshape(8, 128)[PERM, :].copy()
        bv[6:8, :] *= 2.0  # g-gate pre-scale
        brs[d] = np.ascontiguousarray(bv.T)  # [128, 8]
    wtagT = np.ascontiguousarray(np.asarray(W_tag, np.float32).T).astype(
        ml_dtypes.bfloat16)  # [512, 9]
    btag = np.asarray(b_tag, np.float32).reshape(T, 1)
    startv = np.asarray(start_trans, np.float32).reshape(T, 1)
    endv = np.asarray(end_trans, np.float32).reshape(T, 1)
    transm = np.ascontiguousarray(np.asarray(transitions, np.float32))
    transmT = np.ascontiguousarray(transm.T)
    idf32 = np.eye(128, dtype=np.float32)
    idf16 = np.eye(128, dtype=np.float16)
    idq = np.ascontiguousarray(
        np.tile(np.eye(T, dtype=np.float32), (1, 8)).reshape(T, 8, T)
        .transpose(0, 1, 2).reshape(T, 72))

    shared = {
        "emb": embedding, "wih_f": wih["f"], "wih_b": wih["b"],
        "whh_f": whh["f"], "whh_b": whh["b"], "br_f": brs["f"],
        "br_b": brs["b"], "wtagT": wtagT, "btag": btag, "startv": startv,
        "endv": endv, "transm": transm, "transmT": transmT,
        "idf32": idf32, "idf16": idf16, "idq": idq,
    }

    in_maps = []
    tt = np.arange(TOK) // BL   # token -> t
    bb = np.arange(TOK) % BL    # token -> local b
    for c in range(NCORES):
        xc = x[c * BL:(c + 1) * BL]          # [8, 256]
        tc_ = tags[c * BL:(c + 1) * BL]      # [8, 256]
        idx = xc[bb, tt].astype(np.int32)    # [2048] token-major (t,b)
        idx_h = np.ascontiguousarray(idx.reshape(NCH, 128).T)  # [128, NCH]
        tag_tok = tc_[bb, tt]                # [2048]
        ohc = (tag_tok[None, :] == np.arange(T)[:, None]).astype(np.float32)
        nxt = np.full(TOK, -1, np.int64)
        nxt[: TOK - BL] = tag_tok[BL:]       # tag at (t+1, b); t=S-1 -> -1
        ohn = (nxt[None, :] == np.arange(T)[:, None]).astype(np.float32)
        m = dict(shared)
        m["idx"] = idx_h
        m["ohc"] = np.ascontiguousarray(ohc)
        m["ohn"] = np.ascontiguousarray(ohn)
        in_maps.append(m)
    return in_maps


def _run(inputs, trace=False):
    nc = _build(S)
    in_maps = _prep_inputs(**inputs)
    res = run_bass_kernel_spmd(
        nc, in_maps, core_ids=list(range(NCORES)), trace=trace
    )
    total = np.float64(0.0)
    for c in range(NCORES):
        total += np.float64(res.results[c]["out"][0, 0])
    return np.float32(total), res


def kernel(**inputs) -> np.ndarray:
    out, _ = _run(inputs, trace=False)
    return out


# revision 49
# speedup vs baseline: 1.1930x; 1.1930x over previous
"""BiLSTM-CRF NLL kernel for 8 Trainium2 NeuronCores.

Contract: kernel(**inputs) takes the FULL unsharded inputs (as produced by the
reference setup_inputs()) and returns the FULL output (a float32 scalar).

Sharding strategy (hardcoded): data-parallel over the batch dim. B=64 is split
into 8 shards of 8 sequences; LSTM/CRF parameters are replicated on every core.
Each core computes the total NLL of its 8 sequences on-device; the host sums
the 8 partial scalars (the "unshard" step).

Per-core pipeline (all on device):
  0. embedding gather via indirect DMA (token-major [128, E] tiles),
     PE transposes to xT [E, tokens]
  1. input projections g_ih = W_ih @ x + b for all tokens as dense matmuls
     (f32), stored fp16 in SBUF, gate chunks pre-permuted to (i,i,f,f,o,o,g,g)
  2. the two LSTM recurrences (fwd / bwd), interleaved with a half-step skew.
     Per step: 16 bf16 [128,128] weight tiles x [128,8] h -> PSUM [128,64],
     plus one identity-matmul that accumulates the precomputed g_ih into the
     same PSUM bank.  The whole nonlinear tail runs as FOUR back-to-back DVE
     instructions (two custom ops):
       SIG5:  s' = sig~(ps) - 0.5      (deg-5 odd poly + clamp, all 4 gates;
                                        g-gate pre-scaled x2 so s'_g=tanh(g)/2)
       stt:   [u|v|so] = (s'_{i,f,o}+0.5) * [g'|c_old|1]
       stt:   c_new    = 2u + v
       PHM:   h        = tanh~(c_new) * so   (deg-3 odd poly + clamp)
     No ACT/GPSIMD in the chain -> the serial tail latency drops ~2x, and the
     f/b skew hides each direction's tail under the other's matmul group.
  3. emissions transposed [9, tokens] = W_tag.T-chunks @ h, E = exp(emis - mu)
  4. CRF in exp space: the forward-algorithm logsumexp becomes
     A' = (exp(trans).T @ A) * E_t  -- a [9,9]x[9,8] matmul plus one
     elementwise multiply per step. Meet-in-the-middle: a forward chain
     (t=0..127) and a backward chain (t=255..128) run concurrently, halving
     the sequential depth; logZ = log(sum_i A_127 * B_127) + 256*mu.
  5. gold path score via one-hot tensors (host-encoded from tags) and
     matmuls/reductions; output = sum_b (logZ_b - score_b) as [1,1] f32.
"""

import functools
import math
import os
import sys

import numpy as np

for _p in ("/opt/trn_rl_repo", "/opt/pypackages"):
    if _p not in sys.path and os.path.isdir(_p):
        sys.path.append(_p)

import ml_dtypes  # noqa: E402

import concourse.bass as bass  # noqa: E402
import concourse.mybir as mybir  # noqa: E402
import concourse.tile as tile  # noqa: E402
from concourse import bacc  # noqa: E402
from concourse.bass import IndirectOffsetOnAxis  # noqa: E402
from concourse.bass_utils import run_bass_kernel_spmd  # noqa: E402

F32 = mybir.dt.float32
F16 = mybir.dt.float16
BF16 = mybir.dt.bfloat16
I32 = mybir.dt.int32
AF = mybir.ActivationFunctionType
OP = mybir.AluOpType

# Problem constants (hardcoded per the task contract).
B, S, V, E, H, T = 64, 256, 50000, 256, 512, 9
HD = H // 2               # 256 per-direction hidden
NCORES = 8
BL = B // NCORES          # 8 sequences per core
TOK = BL * S              # 2048 tokens per core
NCH = TOK // 128          # 16 gather chunks of 128 tokens
MU = math.log(9.0)        # exp-space drift compensation, cancels exactly
# gate chunk permutation: original (i0 i1 f0 f1 g0 g1 o0 o1) -> (i i f f o o g g)
PERM = [0, 1, 2, 3, 6, 7, 4, 5]
HSLOT = 16                # one h slot = 2 hd-chunks x 8 batch

# --- polynomial activation approximations (fit offline, minimax-ish) -------
# sig~(x)-0.5 = z*(A0 + z^2*(A1 + A2*z^2)), z = clamp(x, +-SIG_B)
SIG_B = 4.2966
SIG_A0 = 0.23636622
SIG_A1 = -0.0121209
SIG_A2 = 0.00030062
# tanh~(x) = z*(T0 + T1*z^2), z = clamp(x, +-TANH_B)
TANH_B = 1.5789
TANH_T0 = 0.89413763
TANH_T1 = -0.12256313


def _register_dve_ops():
    """Register the two LSTM-tail custom DVE ops (idempotent)."""
    from concourse import dve_ops as dops
    from concourse.dve_ops import (DveOp, DveOpSpec, get_dve_sub_opcode,
                                   has_src1)
    from concourse.dve_spec import (  # noqa: F401
        Spec, Src0, Src1, C0, C1, C2, C3, Zero, maxx, minn, sq, lower,
        _spill_c3_to_src1,
    )

    def reg(name, spec):
        for op in dops.OPS:
            if op.name == name:
                return op
        op = DveOp(name, spec, subdim=False, uops_sha={})
        dops.OPS.append(op)
        dops.CUSTOM_DVE_SPECS[name] = spec
        dops._SUB_OPCODE_FOR_NAME[name] = (
            dops._CUSTOM_DVE_ROW_BASE + len(dops.OPS) - 1
        )
        for ver in ("v3", "v4"):
            try:
                compiled = DveOpSpec(
                    name=name, opcode=get_dve_sub_opcode(name),
                    uops=lower(spec, ver=ver), rd1_en=has_src1(spec),
                )
                op.uops_sha[ver] = compiled.sha(ver)
            except Exception:
                pass
        return op

    # SIG5: out = z*(s1 + z2*(imm2 + c3*z2)), z = clamp(in0, +-s0); c3 via in1
    # (minn first so the hoisted Zero-C0 latch-read lands deeper than stage 0)
    z = maxx(minn(Src0, C0), Zero - C0)
    z2 = sq(z)
    w = ((z2 * C3) + C2) * z2 + C1
    sig_body = _spill_c3_to_src1(w * z)

    def sig_ref(in0, in1, s0, s1, imm2):
        zz = np.clip(in0, -s0, s0)
        return (zz * (s1 + zz * zz * (imm2 + in1 * zz * zz))).astype(np.float32)

    sig5 = reg("LSTM_SIG5_ANT", Spec(body=sig_body, reference=sig_ref))

    # PHM: out = in1 * z*(s1 + imm2*z^2), z = clamp(in0, +-s0)
    zp = maxx(minn(Src0, C0), Zero - C0)
    tp = ((sq(zp) * C2) + C1) * zp
    phm_body = tp * Src1

    def phm_ref(in0, in1, s0, s1, imm2):
        zz = np.clip(in0, -s0, s0)
        return (in1 * zz * (s1 + imm2 * zz * zz)).astype(np.float32)

    phm = reg("LSTM_PHM_ANT", Spec(body=phm_body, reference=phm_ref))

    # SIG3T: out = z*(s1 + imm2*z^2), z = clamp(in0 + in1, +-s0).
    # in1 carries the precomputed g_ih tile, so no PSUM preload matmul is
    # needed: the W_hh matmuls alone produce in0.
    z3 = maxx(minn(Src0 + Src1, C0), Zero - C0)
    sig3_body = ((sq(z3) * C2) + C1) * z3

    def sig3_ref(in0, in1, s0, s1, imm2):
        zz = np.clip(in0 + in1, -s0, s0)
        return (zz * (s1 + imm2 * zz * zz)).astype(np.float32)

    sig3 = reg("LSTM_SIG3T_ANT", Spec(body=sig3_body, reference=sig3_ref))
    return sig5, phm, sig3


SIG5_OP, PHM_OP, SIG3_OP = _register_dve_ops()

# deg-3 sigmoid coefficients (fit on [0, 8.3], clamp at SIG3_B)
SIG3_B = 3.3590
SIG3_A0 = 0.21648028
SIG3_A1 = -0.00647874


def _emit_preload(nc, d, t, gih, idf16, ps_pool):
    """Start step-t PSUM with g_ih (+bias) via identity matmul (h-independent)."""
    ps = ps_pool[d].tile([128, 64], F32, tag=f"st{d}", name=f"ps{d}")
    nc.tensor.matmul(
        out=ps[:, :],
        lhsT=idf16[:],
        rhs=gih[d][:, t * 64:(t + 1) * 64],
        start=True,
        stop=False,
        skip_group_check=True,
    )
    return ps


def _emit_wmms(nc, d, t, ps, whh, hall):
    rd = t if d == "f" else t + 1
    for m in range(8):
        for k in range(2):
            nc.tensor.matmul(
                out=ps[:, m * 8:(m + 1) * 8],
                lhsT=whh[d][k][:, m * 128:(m + 1) * 128],
                rhs=hall[d][:, rd * HSLOT + k * 8: rd * HSLOT + k * 8 + 8],
                start=False,
                stop=(m == 7 and k == 1),
                skip_group_check=True,
            )


def _emit_tail_merged(nc, t, ps_cur, hall, arena, work, a2t, sig_only=None):
    """LSTM nonlinear tail for BOTH directions as 6 DVE instructions.

    Arena layout per parity buffer [128, 192]: direction d at base D (f=0,
    b=96): [D:D+64] sig outputs, [D+64:D+80] c state, [D+80:D+96] const 1.
    The cell-update stt ops cover both directions at once through
    [128, 2, 48]-strided views.
    """
    k, k2 = t % 2, (t + 1) % 2

    def tail_one(d, base):
        cur, nxt = arena[k], arena[k2]
        nc.vector._custom_dve(
            SIG5_OP, out=cur[:, base:base + 64], in0=ps_cur[d][:, :],
            in1=a2t[:, 0:1], s0=SIG_B, s1=SIG_A0, imm2=SIG_A1,
        )
        uvo = work.tile([128, 48], F32, tag=f"uvo{d}", name=f"uvo{d}")
        nc.vector.scalar_tensor_tensor(
            uvo[:], cur[:, base:base + 48], 0.5,
            cur[:, base + 48:base + 96], op0=OP.add, op1=OP.mult,
        )
        nc.vector.scalar_tensor_tensor(
            nxt[:, base + 64:base + 80], uvo[:, 0:16], 2.0, uvo[:, 16:32],
            op0=OP.mult, op1=OP.add,
        )
        wr = t + 1 if d == "f" else S - 1 - t
        nc.vector._custom_dve(
            PHM_OP, out=hall[d][:, wr * HSLOT:(wr + 1) * HSLOT],
            in0=nxt[:, base + 64:base + 80], in1=uvo[:, 32:48],
            s0=TANH_B, s1=TANH_T0, imm2=TANH_T1,
        )

    if sig_only == "f":
        tail_one("f", 0)
        return
    tail_one("b", 96)


@functools.lru_cache(maxsize=2)
def _build(seq_len=S):
    """Build the Bass program (same SPMD program for all 8 cores)."""
    assert seq_len == S, "builder is specialized to S=256"

    nc = bacc.Bacc("TRN2", target_bir_lowering=False, debug=False)

    # ---- DRAM I/O ----
    emb_d = nc.dram_tensor("emb", [V, E], F32, kind="ExternalInput")
    idx_d = nc.dram_tensor("idx", [128, NCH], I32, kind="ExternalInput")
    wih_d = {d: nc.dram_tensor(f"wih_{d}", [E, 4 * HD], F32, kind="ExternalInput")
             for d in "fb"}
    whh_d = {d: nc.dram_tensor(f"whh_{d}", [HD, 4 * HD], BF16, kind="ExternalInput")
             for d in "fb"}
    br_d = {d: nc.dram_tensor(f"br_{d}", [128, 8], F32, kind="ExternalInput")
            for d in "fb"}
    wtag_d = nc.dram_tensor("wtagT", [H, T], BF16, kind="ExternalInput")
    btag_d = nc.dram_tensor("btag", [T, 1], F32, kind="ExternalInput")
    start_d = nc.dram_tensor("startv", [T, 1], F32, kind="ExternalInput")
    end_d = nc.dram_tensor("endv", [T, 1], F32, kind="ExternalInput")
    trans_d = nc.dram_tensor("transm", [T, T], F32, kind="ExternalInput")
    transT_d = nc.dram_tensor("transmT", [T, T], F32, kind="ExternalInput")
    ohc_d = nc.dram_tensor("ohc", [T, TOK], F32, kind="ExternalInput")
    ohn_d = nc.dram_tensor("ohn", [T, TOK], F32, kind="ExternalInput")
    idf32_d = nc.dram_tensor("idf32", [128, 128], F32, kind="ExternalInput")
    idf16_d = nc.dram_tensor("idf16", [128, 128], F16, kind="ExternalInput")
    idq_d = nc.dram_tensor("idq", [T, 72], F32, kind="ExternalInput")
    out_d = nc.dram_tensor("out", [1, 1], F32, kind="ExternalOutput")

    with tile.TileContext(nc) as tc:
        with (
            tc.tile_pool(name="pers", bufs=1) as pers,
            tc.tile_pool(name="work", bufs=3) as work,
            tc.tile_pool(name="psbig", bufs=2, space="PSUM") as ps_big,
            tc.tile_pool(name="pstp", bufs=2, space="PSUM") as ps_tp,
            tc.tile_pool(name="psf", bufs=2, space="PSUM") as ps_f,
            tc.tile_pool(name="psb", bufs=2, space="PSUM") as ps_b,
        ):
            ps_pool = {"f": ps_f, "b": ps_b}

            # ---- persistent SBUF ----
            idx_sb = pers.tile([128, NCH], I32, tag="idx")
            nc.sync.dma_start(idx_sb[:], idx_d[:])
            idf32 = pers.tile([128, 128], F32, tag="idf32")
            nc.sync.dma_start(idf32[:], idf32_d[:])
            idf16 = pers.tile([128, 128], F16, tag="idf16")
            nc.sync.dma_start(idf16[:], idf16_d[:])
            a2t = pers.tile([128, 1], F32, tag="a2t")
            nc.vector.memset(a2t[:], SIG_A2)
            a2b = pers.tile([128, 1], F32, tag="a2b")
            nc.vector.memset(a2b[:], SIG_A2)

            wih, whh, br, gih, hall = {}, {}, {}, {}, {}
            for d in "fb":
                wih[d] = [pers.tile([128, 4 * HD], F32, tag=f"wih{d}{k}",
                                    name=f"wih{d}{k}") for k in range(2)]
                for k in range(2):
                    nc.sync.dma_start(wih[d][k][:], wih_d[d][k * 128:(k + 1) * 128, :])
                whh[d] = [pers.tile([128, 4 * HD], BF16, tag=f"whh{d}{k}",
                                    name=f"whh{d}{k}") for k in range(2)]
                for k in range(2):
                    nc.sync.dma_start(whh[d][k][:], whh_d[d][k * 128:(k + 1) * 128, :])
                br[d] = pers.tile([128, 8], F32, tag=f"br{d}", name=f"br{d}")
                nc.sync.dma_start(br[d][:], br_d[d][:])
                gih[d] = pers.tile([128, S * 64], F16, tag=f"gih{d}", name=f"gih{d}")
                hall[d] = pers.tile([128, (S + 1) * HSLOT], BF16, tag=f"hall{d}", name=f"hall{d}")
            # merged tail arenas (both directions in one tile, 2 parities)
            arena = [pers.tile([128, 192], F32, tag=f"arm{k}", name=f"arm{k}")
                     for k in range(2)]
            for base in (0, 96):
                nc.vector.memset(arena[0][:, base + 64:base + 80], 0.0)
                nc.vector.memset(arena[0][:, base + 80:base + 96], 1.0)
                nc.vector.memset(arena[1][:, base + 80:base + 96], 1.0)

            # zero initial h slots (fwd reads slot 0, bwd reads slot S)
            nc.vector.memset(hall["f"][:, 0:HSLOT], 0.0)
            nc.vector.memset(hall["b"][:, S * HSLOT:(S + 1) * HSLOT], 0.0)

            wtagT = [pers.tile([128, T], BF16, tag=f"wtag{kk}", name=f"wtag{kk}")
                      for kk in range(4)]
            for kk in range(4):
                nc.sync.dma_start(wtagT[kk][:], wtag_d[kk * 128:(kk + 1) * 128, :])
            btag = pers.tile([T, 1], F32, tag="btag")
            nc.sync.dma_start(btag[:], btag_d[:])
            startv = pers.tile([T, 1], F32, tag="startv")
            nc.sync.dma_start(startv[:], start_d[:])
            endv = pers.tile([T, 1], F32, tag="endv")
            nc.sync.dma_start(endv[:], end_d[:])
            transm = pers.tile([T, T], F32, tag="transm")
            nc.sync.dma_start(transm[:], trans_d[:])
            transmT = pers.tile([T, T], F32, tag="transmT")
            nc.sync.dma_start(transmT[:], transT_d[:])
            ohc = pers.tile([T, TOK], F32, tag="ohc")
            nc.sync.dma_start(ohc[:], ohc_d[:])
            ohn = pers.tile([T, TOK], F32, tag="ohn")
            nc.sync.dma_start(ohn[:], ohn_d[:])
            ones9 = pers.tile([T, 1], F32, tag="ones9")
            nc.vector.memset(ones9[:], 1.0)
            ones98 = pers.tile([T, 8], F32, tag="ones98")
            nc.vector.memset(ones98[:], 1.0)
            idq = pers.tile([T, 72], F32, tag="idq")
            nc.sync.dma_start(idq[:], idq_d[:])

            # ---- phase 0: gather all chunks up-front (one serial DMA queue,
            # interleaved fwd/bwd order); transposes + per-chunk phase-1 are
            # emitted INSIDE the step loop so the PE FIFO never blocks on a
            # late gather.
            xg = pers.tile([128, NCH * E], F32, tag="xg")
            xT = [pers.tile([128, TOK], F32, tag=f"xT{k}", name=f"xT{k}")
                  for k in range(2)]
            gorder = []
            for j in range(NCH // 2):
                gorder += [j, NCH - 1 - j]
            for ch in gorder:
                nc.gpsimd.indirect_dma_start(
                    out=xg[:, ch * E:(ch + 1) * E],
                    out_offset=None,
                    in_=emb_d[:],
                    in_offset=IndirectOffsetOnAxis(ap=idx_sb[:, ch:ch + 1], axis=0),
                )

            transposed = set()

            def emit_transpose(ch):
                if ch in transposed:
                    return
                transposed.add(ch)
                for k in range(2):
                    pst = ps_tp.tile([128, 128], F32, tag="tp", name="tp")
                    nc.tensor.transpose(
                        out=pst[:],
                        in_=xg[:, ch * E + k * 128: ch * E + (k + 1) * 128],
                        identity=idf32[:],
                    )
                    nc.scalar.copy(xT[k][:, ch * 128:(ch + 1) * 128], pst[:])

            def emit_phase1(d, ch):
                # input projections for one 128-token chunk of direction d.
                # All bias-add/copy ops routed OFF the vector engine (it is
                # reserved for the recurrence tail chain).
                emit_transpose(ch)
                for m in range(8):
                    psg = ps_big.tile([128, 128], F32, tag="big", name="psg")
                    for k in range(2):
                        nc.tensor.matmul(
                            out=psg[:],
                            lhsT=wih[d][k][:, m * 128:(m + 1) * 128],
                            rhs=xT[k][:, ch * 128:(ch + 1) * 128],
                            start=(k == 0),
                            stop=(k == 1),
                        )
                    dst = gih[d][:].rearrange(
                        "p (t m b) -> p t m b", t=S, m=8, b=8
                    )[:, ch * 16:(ch + 1) * 16, m, :]
                    srcv = psg[:].rearrange("p (t b) -> p t b", t=16, b=8)
                    nc.scalar.activation(dst, srcv, AF.Identity,
                                         bias=br[d][:, m:m + 1])

            # ---- phase 1+2 interleaved, with a half-step skew between the
            # two directions: direction f's tail (DVE) executes while
            # direction b's W-matmuls run on the PE, and vice versa.
            emit_phase1("f", 0)
            emit_phase1("b", NCH - 1)
            ps_cur = {"f": _emit_preload(nc, "f", 0, gih, idf16, ps_pool),
                      "b": _emit_preload(nc, "b", S - 1, gih, idf16, ps_pool)}
            _emit_wmms(nc, "f", 0, ps_cur["f"], whh, hall)
            for t in range(S):
                if t == 8:
                    emit_phase1("f", 1)
                elif t >= 16 and t % 16 == 0:
                    q = t // 16
                    if q + 1 < NCH:
                        emit_phase1("f", q + 1)
                tokb = S - 1 - t
                _emit_tail_merged(nc, t, ps_cur, hall, arena, work, a2t,
                                  sig_only="f")
                _emit_wmms(nc, "b", tokb, ps_cur["b"], whh, hall)
                if t + 1 < S:
                    ps_nf = _emit_preload(nc, "f", t + 1, gih, idf16, ps_pool)
                if t == 8:
                    emit_phase1("b", NCH - 2)
                elif t >= 16 and t % 16 == 0:
                    q = t // 16
                    if NCH - 2 - q >= 0:
                        emit_phase1("b", NCH - 2 - q)
                _emit_tail_merged(nc, t, ps_cur, hall, arena, work, a2t)
                if t + 1 < S:
                    ps_nb = _emit_preload(nc, "b", S - 2 - t, gih, idf16,
                                          ps_pool)
                    _emit_wmms(nc, "f", t + 1, ps_nf, whh, hall)
                    ps_cur = {"f": ps_nf, "b": ps_nb}

            # ---- phase 3: emissions (transposed) + E = exp(emis - mu) ----
            emisraw = pers.tile([T, TOK], F32, tag="emisraw")
            ebuf = pers.tile([T, TOK], F32, tag="ebuf")
            hview = {d: hall[d][:].rearrange("p (s c b) -> p s c b", s=S + 1, c=2, b=8)
                     for d in "fb"}
            for n in (1, 2, 0, 3):
                pse = ps_big.tile([T, 512], F32, tag="big")
                for kk in range(4):
                    d = "f" if kk < 2 else "b"
                    c = kk % 2
                    lo = n * 64 + (1 if d == "f" else 0)
                    rhs = hview[d][:, lo:lo + 64, c, :]
                    nc.tensor.matmul(
                        out=pse[:],
                        lhsT=wtagT[kk][:],
                        rhs=rhs,
                        start=(kk == 0),
                        stop=(kk == 3),
                    )
                nc.vector.tensor_scalar_add(
                    emisraw[:, n * 512:(n + 1) * 512], pse[:], btag[:, 0:1]
                )
            negmu = pers.tile([T, 1], F32, tag="negmu")
            nc.vector.memset(negmu[:], -MU)
            nc.scalar.activation(ebuf[:], emisraw[:], AF.Exp, bias=negmu[:, 0:1])

            # ---- phase 4: gold path score ----
            tmp9 = pers.tile([T, TOK], F32, tag="tmp9")
            nc.vector.tensor_tensor(tmp9[:], emisraw[:], ohc[:], op=OP.mult)
            gm = pers.tile([T, 8], F32, tag="gm")
            nc.vector.tensor_reduce(
                gm[:],
                tmp9[:].rearrange("p (t b) -> p b t", t=S, b=8),
                axis=mybir.AxisListType.X,
                op=OP.add,
            )
            for n in range(4):
                psg2 = ps_big.tile([T, 512], F32, tag="big")
                nc.tensor.matmul(
                    out=psg2[:],
                    lhsT=transm[:],
                    rhs=ohc[:, n * 512:(n + 1) * 512],
                    start=True,
                    stop=True,
                )
                nc.vector.tensor_tensor(
                    tmp9[:, n * 512:(n + 1) * 512], psg2[:],
                    ohn[:, n * 512:(n + 1) * 512], op=OP.mult,
                )
            gtr = pers.tile([T, 8], F32, tag="gtr")
            nc.vector.tensor_reduce(
                gtr[:],
                tmp9[:].rearrange("p (t b) -> p b t", t=S, b=8),
                axis=mybir.AxisListType.X,
                op=OP.add,
            )
            gse = pers.tile([T, 8], F32, tag="gse")
            nc.vector.tensor_scalar(
                gse[:], ohc[:, 0:8], scalar1=startv[:, 0:1], scalar2=None,
                op0=OP.mult,
            )
            gee = pers.tile([T, 8], F32, tag="gee")
            nc.vector.tensor_scalar(
                gee[:], ohc[:, (S - 1) * 8:S * 8], scalar1=endv[:, 0:1],
                scalar2=None, op0=OP.mult,
            )
            nc.vector.tensor_tensor(gm[:], gm[:], gtr[:], op=OP.add)
            nc.vector.tensor_tensor(gse[:], gse[:], gee[:], op=OP.add)
            nc.vector.tensor_tensor(gm[:], gm[:], gse[:], op=OP.add)
            ps_sc = ps_tp.tile([1, 8], F32, tag="tp")
            nc.tensor.matmul(out=ps_sc[:], lhsT=ones9[:], rhs=gm[:],
                             start=True, stop=True)
            score_sb = pers.tile([1, 8], F32, tag="score")
            nc.vector.tensor_copy(score_sb[:], ps_sc[:])

            # ---- phase 5: CRF forward/backward exp-space chains ----
            expT = pers.tile([T, T], F32, tag="expT")
            nc.scalar.activation(expT[:], transm[:], AF.Exp)
            expTT = pers.tile([T, T], F32, tag="expTT")
            nc.scalar.activation(expTT[:], transmT[:], AF.Exp)
            exps = pers.tile([T, 1], F32, tag="exps")
            nc.scalar.activation(exps[:], startv[:], AF.Exp)
            expe = pers.tile([T, 1], F32, tag="expe")
            nc.scalar.activation(expe[:], endv[:], AF.Exp)

            # 4-chain scan: cols 0:8 = A (fwd from t=0), 8:16 = B (bwd from
            # t=255), 16:88 = X (identity-init, fwd t=65..127), 88:160 = Y
            # (identity-init, bwd t=190..128).  X/Y columns are b-major
            # (col = 16/88 + b*9 + i).  63 iterations instead of 127; the
            # quarter-chains are stitched with PE-transpose + block-diag
            # matmuls afterwards.
            e3 = ebuf[:].rearrange("p (t b) -> p t b", t=S, b=8)
            tmpAB = work.tile([T, 16], F32, tag="tmpAB")
            nc.vector.tensor_scalar(
                tmpAB[:, 0:8], ebuf[:, 0:8], scalar1=exps[:, 0:1], scalar2=None,
                op0=OP.mult,
            )
            nc.vector.tensor_scalar(
                tmpAB[:, 8:16], ebuf[:, (S - 1) * 8:S * 8],
                scalar1=expe[:, 0:1], scalar2=None, op0=OP.mult,
            )
            # identity init for X and Y: I9 replicated per sequence (host input)
            tmpX = work.tile([T, 72], F32, tag="tmpX")
            tmpY = work.tile([T, 72], F32, tag="tmpY")
            nc.vector.tensor_copy(tmpX[:], idq[:])
            nc.vector.tensor_copy(tmpY[:], idq[:])
            NQ = 63
            for i in range(NQ):  # A: t=1..63; B: t=254..192; X: 65..127; Y: 190..128
                tA = 1 + i
                tB = S - 2 - i
                tX = 65 + i
                tY = S - 66 - i
                psAB = ps_f.tile([T, 16], F32, tag="stf", name="psAB")
                nc.tensor.matmul(out=psAB[:, 0:8], lhsT=expT[:],
                                 rhs=tmpAB[:, 0:8], start=True, stop=True)
                nc.tensor.matmul(out=psAB[:, 8:16], lhsT=expTT[:],
                                 rhs=tmpAB[:, 8:16], start=True, stop=True)
                psX = ps_b.tile([T, 72], F32, tag="stb", name="psX")
                nc.tensor.matmul(out=psX[:], lhsT=expT[:],
                                 rhs=tmpX[:], start=True, stop=True)
                psY = ps_big.tile([T, 72], F32, tag="big", name="psY")
                nc.tensor.matmul(out=psY[:], lhsT=expTT[:],
                                 rhs=tmpY[:], start=True, stop=True)
                tmpAB = work.tile([T, 16], F32, tag="tmpAB")
                nc.vector.tensor_tensor(
                    tmpAB[:], psAB[:],
                    e3[:, tA:tB + 1:(tB - tA), :], op=OP.mult,
                )
                tmpX = work.tile([T, 72], F32, tag="tmpX")
                nc.vector.tensor_tensor(
                    tmpX[:].rearrange("p (b i) -> p b i", b=8, i=9),
                    psX[:].rearrange("p (b i) -> p b i", b=8, i=9),
                    e3[:, tX, :].broadcast_to([T, 8, 9]), op=OP.mult,
                )
                tmpY = work.tile([T, 72], F32, tag="tmpY")
                nc.vector.tensor_tensor(
                    tmpY[:].rearrange("p (b i) -> p b i", b=8, i=9),
                    psY[:].rearrange("p (b i) -> p b i", b=8, i=9),
                    e3[:, tY, :].broadcast_to([T, 8, 9]), op=OP.mult,
                )
            # bridges: A_64 = (expT^T A_63) * E_64 ; B_191 = (expTT^T B_192)*E_191
            psbr = ps_f.tile([T, 16], F32, tag="stf", name="psbr")
            nc.tensor.matmul(out=psbr[:, 0:8], lhsT=expT[:],
                             rhs=tmpAB[:, 0:8], start=True, stop=True)
            nc.tensor.matmul(out=psbr[:, 8:16], lhsT=expTT[:],
                             rhs=tmpAB[:, 8:16], start=True, stop=True)
            ab64 = pers.tile([T, 16], F32, tag="ab64")
            nc.vector.tensor_tensor(
                ab64[:], psbr[:],
                e3[:, 64:192:127, :], op=OP.mult,
            )
            # stitch: A_127 = X_blockdiag @ A_64, B_128st = Y_blockdiag @ B_191
            abf = pers.tile([T, 16], F32, tag="abf")
            for side, src in (("x", tmpX), ("y", tmpY)):
                pstx = ps_tp.tile([128, 128], F32, tag="tp", name="tp")
                nc.tensor.transpose(out=pstx[0:72, 0:T],
                                    in_=src[:],
                                    identity=idf32[0:T, 0:T])
                xt = pers.tile([72, T], F32, tag=f"xt{side}")
                nc.vector.tensor_copy(xt[:], pstx[0:72, 0:T])
                bd = pers.tile([72, 8], F32, tag=f"bd{side}")
                nc.vector.memset(bd[:], 0.0)
                co = 0 if side == "x" else 8
                for bq in range(8):
                    nc.sync.dma_start(bd[bq * 9:bq * 9 + 9, bq:bq + 1],
                                      ab64[:, co + bq:co + bq + 1])
                psf2 = ps_tp.tile([T, 8], F32, tag="tp", name="tpf")
                nc.tensor.matmul(out=psf2[:], lhsT=xt[:], rhs=bd[:],
                                 start=True, stop=True)
                nc.vector.tensor_copy(abf[:, co:co + 8], psf2[:])
            # final B matmul: bridge the middle transition t=128
            psB = ps_b.tile([T, 8], F32, tag="stb")
            nc.tensor.matmul(out=psB[:], lhsT=expTT[:], rhs=abf[:, 8:16],
                             start=True, stop=True)
            ab = work.tile([T, 8], F32, tag="ab")
            nc.vector.tensor_tensor(ab[:], abf[:, 0:8], psB[:], op=OP.mult)
            psZ = ps_tp.tile([1, 8], F32, tag="tp")
            nc.tensor.matmul(out=psZ[:], lhsT=ones9[:], rhs=ab[:],
                             start=True, stop=True)
            lz = pers.tile([1, 8], F32, tag="lz")
            nc.scalar.activation(lz[:], psZ[:], AF.Ln)
            diff = pers.tile([1, 8], F32, tag="diff")
            nc.vector.tensor_tensor(diff[:], lz[:], score_sb[:], op=OP.subtract)
            red = pers.tile([1, 1], F32, tag="red")
            nc.vector.tensor_reduce(red[:], diff[:], axis=mybir.AxisListType.X,
                                    op=OP.add)
            outc = pers.tile([1, 1], F32, tag="outc")
            nc.vector.tensor_scalar_add(outc[:], red[:], float(BL * S * MU))
            nc.sync.dma_start(out_d[:], outc[:])

    nc.finalize()
    return nc


def _prep_inputs(x, tags, crf_mask, embedding, W_ih_f, W_hh_f, b_f, W_ih_b,
                 W_hh_b, b_b, W_tag, b_tag, transitions, start_trans, end_trans):
    """Host-side sharding + layout prep. Pure reformatting / dtype casts."""
    x = np.asarray(x).astype(np.int32)
    tags = np.asarray(tags).astype(np.int32)
    mask = np.asarray(crf_mask)
    assert mask.all(), "kernel specialized to all-ones crf_mask"
    embedding = np.ascontiguousarray(np.asarray(embedding, dtype=np.float32))

    def perm_cols(w):  # [*, 4HD] -> gate-chunk permuted cols, g-gate x2
        wc = w.reshape(w.shape[0], 8, 128)[:, PERM, :].copy()
        wc[:, 6:8, :] *= 2.0  # g-gate pre-scale: tanh(g) = 2*sigmoid(2g) - 1
        return np.ascontiguousarray(wc.reshape(w.shape[0], 4 * HD))

    wih = {"f": perm_cols(np.asarray(W_ih_f, np.float32).T),
           "b": perm_cols(np.asarray(W_ih_b, np.float32).T)}
    whh = {"f": perm_cols(np.asarray(W_hh_f, np.float32).T).astype(ml_dtypes.bfloat16),
           "b": perm_cols(np.asarray(W_hh_b, np.float32).T).astype(ml_dtypes.bfloat16)}
    brs = {}
    for d, b_ in (("f", b_f), ("b", b_b)):
        bv = np.asarray(b_, np.float32).reshape(8, 128)[PERM, :].copy()
        bv[6:8, :] *= 2.0  # g-gate pre-scale
        brs[d] = np.ascontiguousarray(bv.T)  # [128, 8]
    wtagT = np.ascontiguousarray(np.asarray(W_tag, np.float32).T).astype(
        ml_dtypes.bfloat16)  # [512, 9]
    btag = np.asarray(b_tag, np.float32).reshape(T, 1)
    startv = np.asarray(start_trans, np.float32).reshape(T, 1)
    endv = np.asarray(end_trans, np.float32).reshape(T, 1)
    transm = np.ascontiguousarray(np.asarray(transitions, np.float32))
    transmT = np.ascontiguousarray(transm.T)
    idf32 = np.eye(128, dtype=np.float32)
    idf16 = np.eye(128, dtype=np.float16)
    idq = np.ascontiguousarray(
        np.tile(np.eye(T, dtype=np.float32), (1, 8)).reshape(T, 8, T)
        .transpose(0, 1, 2).reshape(T, 72))

    shared = {
        "emb": embedding, "wih_f": wih["f"], "wih_b": wih["b"],
        "whh_f": whh["f"], "whh_b": whh["b"], "br_f": brs["f"],
        "br_b": brs["b"], "wtagT": wtagT, "btag": btag, "startv": startv,
        "endv": endv, "transm": transm, "transmT": transmT,
        "idf32": idf32, "idf16": idf16, "idq": idq,
    }

    in_maps = []
    tt = np.arange(TOK) // BL   # token -> t
    bb = np.arange(TOK) % BL    # token -> local b
    for c in range(NCORES):
        xc = x[c * BL:(c + 1) * BL]          # [8, 256]
        tc_ = tags[c * BL:(c + 1) * BL]      # [8, 256]
        idx = xc[bb, tt].astype(np.int32)    # [2048] token-major (t,b)
        idx_h = np.ascontiguousarray(idx.reshape(NCH, 128).T)  # [128, NCH]
        tag_tok = tc_[bb, tt]                # [2048]
        ohc = (tag_tok[None, :] == np.arange(T)[:, None]).astype(np.float32)
        nxt = np.full(TOK, -1, np.int64)
        nxt[: TOK - BL] = tag_tok[BL:]       # tag at (t+1, b); t=S-1 -> -1
        ohn = (nxt[None, :] == np.arange(T)[:, None]).astype(np.float32)
        m = dict(shared)
        m["idx"] = idx_h
        m["ohc"] = np.ascontiguousarray(ohc)
        m["ohn"] = np.ascontiguousarray(ohn)
        in_maps.append(m)
    return in_maps


def _run(inputs, trace=False):
    nc = _build(S)
    in_maps = _prep_inputs(**inputs)
    res = run_bass_kernel_spmd(
        nc, in_maps, core_ids=list(range(NCORES)), trace=trace
    )
    total = np.float64(0.0)
    for c in range(NCORES):
        total += np.float64(res.results[c]["out"][0, 0])
    return np.float32(total), res


def kernel(**inputs) -> np.ndarray:
    out, _ = _run(inputs, trace=False)
    return out
